# revision 2
# baseline (speedup 1.0000x reference)
"""Multi-head self-attention Trainium2 kernel (tensor-parallel over heads x batch).

Sharding: 8 cores; core c handles batch b=c//4, heads h0=(c%4)*4 .. h0+3.
Each core computes its 4 heads' attention weights [4,2048,2048] and a partial
output projection [2048,1024]; host sums partials per batch and adds bo.

Emission order interleaves phase-A halves, score/AV passes (B2), weight
passes (B1) and the output projection so that ACT/PE/DMA overlap.
"""
import numpy as np
import concourse.bass as bass
import concourse.mybir as mybir
import concourse.tile as tile
from concourse import bacc
from concourse.hw_specs import get_activation_tables

F32 = mybir.dt.float32
FR = mybir.dt.float32r
AF = mybir.ActivationFunctionType
ALU = mybir.AluOpType

B, T, D, H, DK = 2, 2048, 1024, 16, 64
HPC = 4            # heads per core
CD = HPC * DK      # local head-dim total = 256
NT = T // 128      # 16 t-tiles
NCH = T // 512     # 4 t-chunks


def R(ap):
    return ap.bitcast(FR)


def build_nc(n_tt=NT, repeat=1):
    nc = bacc.Bacc("TRN2", target_bir_lowering=False, debug=False, num_devices=8)

    xT = nc.dram_tensor("xT", [D, T], FR, kind="ExternalInput")
    wqT = nc.dram_tensor("wqT", [D, CD], FR, kind="ExternalInput")
    wkT = nc.dram_tensor("wkT", [D, CD], FR, kind="ExternalInput")
    wvT = nc.dram_tensor("wvT", [D, CD], FR, kind="ExternalInput")
    woT = nc.dram_tensor("woT", [CD, D], FR, kind="ExternalInput")
    bq = nc.dram_tensor("bq", [2, 128], F32, kind="ExternalInput")  # pre-scaled 1/8
    bk = nc.dram_tensor("bk", [2, 128], F32, kind="ExternalInput")
    bv = nc.dram_tensor("bv", [1, CD], FR, kind="ExternalInput")
    tri = nc.dram_tensor("tri", [128, 128], F32, kind="ExternalInput")
    trit = nc.dram_tensor("trit", [128, 128], FR, kind="ExternalInput")
    idm = nc.dram_tensor("idm", [128, 128], F32, kind="ExternalInput")
    onesv = nc.dram_tensor("onesv", [128, 128], FR, kind="ExternalInput")

    out_w = nc.dram_tensor("out_w", [HPC, T, T], F32, kind="ExternalOutput")
    out_p = nc.dram_tensor("out_p", [T, D], F32, kind="ExternalOutput")

    with tile.TileContext(nc) as tc:
     for _rep in range(repeat):
        with tc.tile_pool(name="persist", bufs=1) as P, \
             tc.tile_pool(name="stats", bufs=2) as SP, \
             tc.tile_pool(name="epool", bufs=4) as EP, \
             tc.tile_pool(name="wpool", bufs=4) as WP, \
             tc.tile_pool(name="opool", bufs=2) as OP, \
             tc.tile_pool(name="psA", bufs=2, space="PSUM") as psA, \
             tc.tile_pool(name="psP", bufs=2, space="PSUM") as psP, \
             tc.tile_pool(name="psAV", bufs=2, space="PSUM") as psAV:

            wq_s = P.tile([128, 8 * CD], FR, tag="wq_s")
            wk_s = P.tile([128, 8 * CD], FR, tag="wk_s")
            wv_s = P.tile([128, 8 * CD], FR, tag="wv_s")
            wo_s = P.tile([128, 2 * D], FR, tag="wo_s")
            bq_s = P.tile([128, 2], F32, tag="bq_s")
            bk_s = P.tile([128, 2], F32, tag="bk_s")
            bv_s = P.tile([1, CD], FR, tag="bv_s")
            tri_s = P.tile([128, 128], F32, tag="tri_s")
            trit_s = P.tile([128, 128], FR, tag="trit_s")
            id_s = P.tile([128, 128], F32, tag="id_s")
            one128 = P.tile([1, 128], FR, tag="one128")
            one64 = P.tile([1, 64], FR, tag="one64")
            qt = [P.tile([128, T], FR, tag=f"qt{m}", name=f"qt{m}") for m in range(2)]
            kt = [P.tile([128, T], FR, tag=f"kt{m}", name=f"kt{m}") for m in range(2)]
            v_s = P.tile([128, NT * 260], FR, tag="v_s")
            hc = [P.tile([128, T], FR, tag=f"hc{m}", name=f"hc{m}") for m in range(2)]
            r_all = [P.tile([128, 16], F32, tag=f"r_all{h}", name=f"r_all{h}")
                     for h in range(HPC)]
            lnr = [P.tile([128, 16], F32, tag=f"lnr{h}", name=f"lnr{h}")
                   for h in range(HPC)]

            dma = nc.sync.dma_start
            act = nc.scalar.activation
            vec = nc.vector

            # pin the combined exp+ln table so Exp<->Ln alternation never reloads
            _tables = get_activation_tables(nc.m.arch)
            for _idx, (_nm, _funcs) in enumerate(_tables.items()):
                if AF.Exp in _funcs and AF.Ln in _funcs:
                    nc.scalar.add_instruction(mybir.InstLoadActFuncSet(
                        name=nc.get_next_instruction_name(), ins=[], outs=[],
                        act_func_set_id=_idx))
                    break

            dma(out=wq_s.rearrange("p (k c) -> p k c", k=8),
                in_=wqT.rearrange("(k p) c -> p k c", p=128))

            def deferred_loads():
                dma(out=trit_s, in_=trit[:, :])
                dma(out=id_s, in_=idm[:, :])
                dma(out=tri_s, in_=tri[:, :])
                dma(out=bq_s, in_=bq.rearrange("m p -> p m"))
                dma(out=bk_s, in_=bk.rearrange("m p -> p m"))
                dma(out=one128, in_=onesv[0:1, :])
                dma(out=one64, in_=onesv[0:1, 0:64])
                dma(out=bv_s, in_=bv[:, :])
                dma(out=v_s.rearrange("p (blk h e) -> p blk h e", blk=NT, h=HPC)[:, :, :, 64:65],
                    in_=onesv[:, 0:64].rearrange("p (a b) -> p a b", a=NT))
                dma(out=wk_s.rearrange("p (k c) -> p k c", k=8),
                    in_=wkT.rearrange("(k p) c -> p k c", p=128))
                dma(out=wv_s.rearrange("p (k c) -> p k c", k=8),
                    in_=wvT.rearrange("(k p) c -> p k c", p=128))
                dma(out=wo_s.rearrange("p (k c) -> p k c", k=2),
                    in_=woT.rearrange("(k p) c -> p k c", p=128))

            TH = T // 2
            XK_cm = tc.tile_pool(name="xk", bufs=1)
            XK = XK_cm.__enter__()


            def a_part(half, part, fillers=()):
                # part 0: loads + Q/K t-chunk 2*half, V s-blocks 8*half..+3
                # part 1: Q/K t-chunk 2*half+1, V s-blocks 8*half+4..+7
                fillers = list(fillers)

                def fill():
                    if fillers:
                        fillers.pop(0)()
                tch = 2 * half + part
                xh = []
                for k in range(8):
                    t_ = XK.tile([128, 512], FR, tag=f"xk{k}",
                                 name=f"xk{k}q{tch}")
                    dma(out=t_, in_=xT[128 * k:128 * (k + 1),
                                       512 * tch: 512 * (tch + 1)])
                    xh.append(t_)
                if half == 0 and part == 0:
                    deferred_loads()
                for (wt, qkt, bias, scale) in ((wq_s, qt, bq_s, 0.125),
                                               (wk_s, kt, bk_s, 1.0)):
                    for m in range(2):
                        ps = psA.tile([128, 512], F32, tag="psA")
                        for k in range(8):
                            nc.tensor.matmul(
                                ps,
                                R(wt[:, 256 * k + 128 * m: 256 * k + 128 * m + 128]),
                                R(xh[k]),
                                start=(k == 0), stop=(k == 7))
                        with nc.allow_low_precision(reason="f32r out"):
                            vec.tensor_scalar(
                                qkt[m][:, 512 * tch: 512 * (tch + 1)], ps,
                                scalar1=scale, scalar2=bias[:, m:m + 1],
                                op0=ALU.mult, op1=ALU.add)
                        fill()
                for b_ in range(4):
                    blk = 4 * tch + b_
                    ps = psA.tile([128, 512], F32, tag="psA")
                    for k in range(8):
                        nc.tensor.matmul(
                            ps[:, 0:CD],
                            R(xh[k][:, 128 * b_: 128 * (b_ + 1)]),
                            R(wv_s[:, 256 * k: 256 * (k + 1)]),
                            start=(k == 0), stop=False)
                    nc.tensor.matmul(ps[:, 0:CD], R(one128), R(bv_s),
                                     start=False, stop=True)
                    vec.tensor_copy(
                        v_s[:, 260 * blk: 260 * blk + 260]
                        .rearrange("p (h e) -> p h e", h=HPC)[:, :, 0:64],
                        ps[:, 0:CD].rearrange("p (h e) -> p h e", e=64))
                    fill()
                while fillers:
                    fillers.pop(0)()

            def b2_chunk(h, c):
                """Scores in [s,t], exp, AV+Z fused; updates hc, r_all, lnr[4c:4c+4]."""
                m, po = h // 2, 64 * (h % 2)
                qh = qt[m][po:po + 64, :]
                kh = kt[m][po:po + 64, :]
                qs = qh[:, 512 * c: 512 * (c + 1)]
                av = psAV.tile([65, 512], F32, tag="psAV", name=f"av{h}_{c}")
                work = []
                # paired full blocks -> one [128,1024] psum + one exp
                for a in range(0, 4 * c, 2):
                    sp = psP.tile([128, 1024], F32, tag="psP", name=f"sp{h}_{c}_{a}")
                    for d_ in range(2):
                        nc.tensor.matmul(
                            sp[:, 512 * d_: 512 * (d_ + 1)],
                            R(kh[:, 128 * (a + d_): 128 * (a + d_ + 1)]),
                            R(qs),
                            start=True, stop=True)
                    e = EP.tile([128, 1024], FR, tag="e2", name=f"e{h}_{c}_{a}")
                    act(e, sp, AF.Exp)
                    work.append((a, e[:, 0:512], 0))
                    work.append((a + 1, e[:, 512:1024], 0))
                # diagonal-region singles, column-trimmed
                for r_ in range(4):
                    blk = 4 * c + r_
                    off = 128 * r_
                    sp = psA.tile([128, 512], F32, tag="psA", name=f"sd{h}_{c}_{r_}")
                    nc.tensor.matmul(
                        sp[:, 0:512 - off],
                        R(kh[:, 128 * blk: 128 * (blk + 1)]),
                        R(qh[:, 512 * c + off: 512 * (c + 1)]),
                        start=True, stop=True)
                    e = EP.tile([128, 512], FR, tag="e", name=f"ed{h}_{c}_{r_}")
                    act(e[:, off:512], sp[:, 0:512 - off], AF.Exp)
                    vec.tensor_mul(e[:, off:off + 128], e[:, off:off + 128],
                                   trit_s)
                    work.append((blk, e, off))
                for idx, (blk, e_, off) in enumerate(work):
                    nc.tensor.matmul(
                        av[0:65, off:512],
                        R(v_s[:, 260 * blk + 65 * h: 260 * blk + 65 * h + 65]),
                        R(e_[:, off:512] if e_.shape[-1] == 512 else e_),
                        start=(idx == 0), stop=(idx == len(work) - 1))

                ztmp = SP.tile([1, 512], F32, tag="ztmp", name=f"zt{h}_{c}")
                vec.tensor_copy(ztmp, av[64:65, :])
                for k in range(4):
                    i_tt = 4 * c + k
                    zt = psA.tile([128, 4], F32, tag="psA", name=f"ztp{h}_{c}_{k}")
                    nc.tensor.matmul(zt[:, 0:1], ztmp[0:1, 128 * k:128 * (k + 1)],
                                     id_s[0:1, 0:1], is_transpose=True,
                                     start=True, stop=True)
                    vec.reciprocal(r_all[h][:, i_tt:i_tt + 1], zt[:, 0:1])
                rrow = SP.tile([1, 512], FR, tag="rrow", name=f"rr{h}_{c}")
                with nc.allow_low_precision(reason="f32r is f32-width"):
                    vec.reciprocal(rrow, av[64:65, :])
                rb_ps = psA.tile([64, 512], F32, tag="psA", name=f"rb{h}_{c}")
                nc.tensor.matmul(rb_ps, R(one64), R(rrow), start=True, stop=True)
                rbc = SP.tile([64, 512], F32, tag="rbc", name=f"rbc{h}_{c}")
                vec.tensor_copy(rbc, rb_ps)
                vec.tensor_mul(
                    hc[m][po:po + 64, 512 * c: 512 * (c + 1)],
                    av[0:64, :], rbc)
                act(lnr[h][:, 4 * c:4 * (c + 1)], r_all[h][:, 4 * c:4 * (c + 1)],
                    AF.Ln)

            def b1_tile(h, i):
                m, po = h // 2, 64 * (h % 2)
                qh = qt[m][po:po + 64, :]
                kh = kt[m][po:po + 64, :]
                ncols = (i + 1) * 128
                w_sb = WP.tile([128, 2048], F32, tag="w", name=f"w{h}_{i}")
                qi = qh[:, 128 * i: 128 * (i + 1)]
                for a in range(0, (ncols + 1023) // 1024):
                    base = 1024 * a
                    M = min(1024, ncols - base)
                    sp = psP.tile([128, 1024], F32, tag="psP", name=f"wp{h}_{i}_{a}")
                    for d_ in range(0, M, 512):
                        N = min(512, M - d_)
                        nc.tensor.matmul(
                            sp[:, d_:d_ + N],
                            R(qi),
                            R(kh[:, base + d_: base + d_ + N]),
                            start=True, stop=True)
                    act(w_sb[:, base:base + M], sp[:, 0:M], AF.Exp,
                        bias=lnr[h][:, i:i + 1], scale=1.0)
                vec.tensor_mul(w_sb[:, ncols - 128:ncols],
                               w_sb[:, ncols - 128:ncols], tri_s)
                dma(out=out_w[h, 128 * i: 128 * (i + 1), 0:ncols],
                    in_=w_sb[:, 0:ncols])

            def proj_chunk(c):
                for q in range(4):
                    o_sb = OP.tile([128, D], F32, tag="o", name=f"o{c}_{q}")
                    op = psP.tile([128, 1024], F32, tag="psP", name=f"op{c}_{q}")
                    for n in range(2):
                        for k in range(2):
                            nc.tensor.matmul(
                                op[:, 512 * n: 512 * (n + 1)],
                                R(hc[k][:, 512 * c + 128 * q: 512 * c + 128 * (q + 1)]),
                                R(wo_s[:, 1024 * k + 512 * n: 1024 * k + 512 * (n + 1)]),
                                start=(k == 0), stop=(k == 1))
                    vec.tensor_copy(o_sb, op)
                    dma(out=out_p[128 * (4 * c + q): 128 * (4 * c + q + 1), :],
                        in_=o_sb)

            # ---------- schedule ----------
            def F(h, i):
                return lambda: b1_tile(h, i)

            a_part(0, 0)
            for h in range(HPC):
                b2_chunk(h, 0)
            a_part(0, 1, fillers=[F(h, i) for i in range(0, 3)
                                  for h in range(HPC)])
            for h in range(HPC):
                b2_chunk(h, 1)
            a_part(1, 0, fillers=[F(h, i) for i in (3, 4, 5)
                                  for h in range(HPC)])
            for h in range(HPC):
                b2_chunk(h, 2)
            a_part(1, 1, fillers=[F(h, i) for i in (6, 7, 8)
                                  for h in range(HPC)])
            XK_cm.__exit__(None, None, None)
            for h in range(HPC):
                b2_chunk(h, 3)
                for i in range(9, 12):
                    b1_tile(h, i)
                if h == 1:
                    proj_chunk(0)
                if h == 2:
                    proj_chunk(1)
            for h in range(HPC):
                for i in range(12, 16):
                    b1_tile(h, i)
                if h == 1:
                    proj_chunk(2)
                if h == 2:
                    proj_chunk(3)

    nc.compile()
    return nc


def make_host_inputs(x, Wq, bq, Wk, bk, Wv, bv, Wo):
    tri = np.tril(np.ones((128, 128), dtype=np.float32))
    idm = np.eye(128, dtype=np.float32)

    in_maps = []
    for c in range(8):
        b = c // 4
        r0 = (c % 4) * CD
        in_maps.append({
            "xT": np.ascontiguousarray(x[b].T),
            "wqT": np.ascontiguousarray(Wq[r0:r0 + CD, :].T),
            "wkT": np.ascontiguousarray(Wk[r0:r0 + CD, :].T),
            "wvT": np.ascontiguousarray(Wv[r0:r0 + CD, :].T),
            "woT": np.ascontiguousarray(Wo[:, r0:r0 + CD].T),
            "bq": (bq[r0:r0 + CD] * 0.125).reshape(2, 128).astype(np.float32),
            "bk": bk[r0:r0 + CD].reshape(2, 128).astype(np.float32),
            "bv": bv[r0:r0 + CD].reshape(1, CD).astype(np.float32),
            "tri": tri, "trit": np.ascontiguousarray(tri.T), "idm": idm,
            "onesv": np.ones((128, 128), dtype=np.float32),
        })
    return in_maps


def assemble(results, bo):
    weights = np.empty((B, H, T, T), dtype=np.float32)
    out = np.zeros((B, T, D), dtype=np.float32)
    for c in range(8):
        b = c // 4
        h0 = (c % 4) * HPC
        weights[b, h0:h0 + HPC] = results[c]["out_w"]
        out[b] += results[c]["out_p"]
    out += bo
    return out, weights


_CACHE = {}


def kernel(x, Wq, bq, Wk, bk, Wv, bv, Wo, bo):
    """Full-input entry: shards across 8 NeuronCores, runs the Bass kernel,
    reassembles full outputs. Returns (out [2,2048,1024], weights [2,16,2048,2048])."""
    from concourse.bass_utils import run_bass_kernel_spmd
    x, Wq, bq, Wk, bk, Wv, bv, Wo, bo = (
        np.asarray(a, dtype=np.float32)
        for a in (x, Wq, bq, Wk, bk, Wv, bv, Wo, bo))
    if "nc" not in _CACHE:
        _CACHE["nc"] = build_nc()
    in_maps = make_host_inputs(x, Wq, bq, Wk, bk, Wv, bv, Wo)
    res = run_bass_kernel_spmd(_CACHE["nc"], in_maps, list(range(8)))
    return assemble(res.results, bo)


# revision 3
# speedup vs baseline: 1.6253x; 1.6253x over previous
"""Multi-head self-attention Trainium2 kernel (tensor-parallel over heads x batch).

Sharding: 8 cores; core c handles batch b=c//4, heads h0=(c%4)*4 .. h0+3.
Each core computes its 4 heads' attention weights [4,2048,2048] and a partial
output projection [2048,1024]; host sums partials per batch and adds bo.

Emission order interleaves phase-A halves, score/AV passes (B2), weight
passes (B1) and the output projection so that ACT/PE/DMA overlap.
"""
import numpy as np
import concourse.bass as bass
import concourse.mybir as mybir
import concourse.tile as tile
from concourse import bacc
from concourse.hw_specs import get_activation_tables

F32 = mybir.dt.float32
FR = mybir.dt.float32r
AF = mybir.ActivationFunctionType
ALU = mybir.AluOpType

B, T, D, H, DK = 2, 2048, 1024, 16, 64
HPC = 4            # heads per core
CD = HPC * DK      # local head-dim total = 256
NT = T // 128      # 16 t-tiles
NCH = T // 512     # 4 t-chunks


def R(ap):
    return ap.bitcast(FR)


def build_nc(n_tt=NT, repeat=1):
    nc = bacc.Bacc("TRN2", target_bir_lowering=False, debug=False, num_devices=8)

    xT = nc.dram_tensor("xT", [D, T], FR, kind="ExternalInput")
    wqT = nc.dram_tensor("wqT", [D, CD], FR, kind="ExternalInput")
    wkT = nc.dram_tensor("wkT", [D, CD], FR, kind="ExternalInput")
    wvT = nc.dram_tensor("wvT", [D, CD], FR, kind="ExternalInput")
    woT = nc.dram_tensor("woT", [CD, D], FR, kind="ExternalInput")
    bq = nc.dram_tensor("bq", [2, 128], F32, kind="ExternalInput")  # pre-scaled 1/8
    bk = nc.dram_tensor("bk", [2, 128], F32, kind="ExternalInput")
    bv = nc.dram_tensor("bv", [1, CD], FR, kind="ExternalInput")
    tri = nc.dram_tensor("tri", [128, 128], F32, kind="ExternalInput")
    trit = nc.dram_tensor("trit", [128, 128], FR, kind="ExternalInput")
    idm = nc.dram_tensor("idm", [128, 128], F32, kind="ExternalInput")
    onesv = nc.dram_tensor("onesv", [128, 128], FR, kind="ExternalInput")

    out_w = nc.dram_tensor("out_w", [HPC, T, T], F32, kind="ExternalOutput")
    out_p = nc.dram_tensor("out_p", [T, D], F32, kind="ExternalOutput")

    with tile.TileContext(nc) as tc:
     for _rep in range(repeat):
        with tc.tile_pool(name="persist", bufs=1) as P, \
             tc.tile_pool(name="stats", bufs=2) as SP, \
             tc.tile_pool(name="epool", bufs=4) as EP, \
             tc.tile_pool(name="wpool", bufs=4) as WP, \
             tc.tile_pool(name="opool", bufs=2) as OP, \
             tc.tile_pool(name="psA", bufs=2, space="PSUM") as psA, \
             tc.tile_pool(name="psP", bufs=2, space="PSUM") as psP, \
             tc.tile_pool(name="psAV", bufs=2, space="PSUM") as psAV:

            wq_s = P.tile([128, 8 * CD], FR, tag="wq_s")
            wk_s = P.tile([128, 8 * CD], FR, tag="wk_s")
            wv_s = P.tile([128, 8 * CD], FR, tag="wv_s")
            wo_s = P.tile([128, 2 * D], FR, tag="wo_s")
            bq_s = P.tile([128, 2], F32, tag="bq_s")
            bk_s = P.tile([128, 2], F32, tag="bk_s")
            bv_s = P.tile([1, CD], FR, tag="bv_s")
            tri_s = P.tile([128, 128], F32, tag="tri_s")
            trit_s = P.tile([128, 128], FR, tag="trit_s")
            id_s = P.tile([128, 128], F32, tag="id_s")
            one128 = P.tile([1, 128], FR, tag="one128")
            one64 = P.tile([1, 64], FR, tag="one64")
            qt = [P.tile([128, T], FR, tag=f"qt{m}", name=f"qt{m}") for m in range(2)]
            kt = [P.tile([128, T], FR, tag=f"kt{m}", name=f"kt{m}") for m in range(2)]
            v_s = P.tile([128, NT * 260], FR, tag="v_s")
            hc = [P.tile([128, T], FR, tag=f"hc{m}", name=f"hc{m}") for m in range(2)]
            r_all = [P.tile([128, 16], F32, tag=f"r_all{h}", name=f"r_all{h}")
                     for h in range(HPC)]
            lnr = [P.tile([128, 16], F32, tag=f"lnr{h}", name=f"lnr{h}")
                   for h in range(HPC)]

            dma = nc.sync.dma_start
            act = nc.scalar.activation
            vec = nc.vector

            # pin the combined exp+ln table so Exp<->Ln alternation never reloads
            _tables = get_activation_tables(nc.m.arch)
            for _idx, (_nm, _funcs) in enumerate(_tables.items()):
                if AF.Exp in _funcs and AF.Ln in _funcs:
                    nc.scalar.add_instruction(mybir.InstLoadActFuncSet(
                        name=nc.get_next_instruction_name(), ins=[], outs=[],
                        act_func_set_id=_idx))
                    break

            dma(out=wq_s.rearrange("p (k c) -> p k c", k=8),
                in_=wqT.rearrange("(k p) c -> p k c", p=128))

            def deferred_loads():
                dma(out=trit_s, in_=trit[:, :])
                dma(out=id_s, in_=idm[:, :])
                dma(out=tri_s, in_=tri[:, :])
                dma(out=bq_s, in_=bq.rearrange("m p -> p m"))
                dma(out=bk_s, in_=bk.rearrange("m p -> p m"))
                dma(out=one128, in_=onesv[0:1, :])
                dma(out=one64, in_=onesv[0:1, 0:64])
                dma(out=bv_s, in_=bv[:, :])
                dma(out=v_s.rearrange("p (blk h e) -> p blk h e", blk=NT, h=HPC)[:, :, :, 64:65],
                    in_=onesv[:, 0:64].rearrange("p (a b) -> p a b", a=NT))
                dma(out=wk_s.rearrange("p (k c) -> p k c", k=8),
                    in_=wkT.rearrange("(k p) c -> p k c", p=128))
                dma(out=wv_s.rearrange("p (k c) -> p k c", k=8),
                    in_=wvT.rearrange("(k p) c -> p k c", p=128))
                dma(out=wo_s.rearrange("p (k c) -> p k c", k=2),
                    in_=woT.rearrange("(k p) c -> p k c", p=128))

            TH = T // 2
            XK_cm = tc.tile_pool(name="xk", bufs=1)
            XK = XK_cm.__enter__()


            def a_part(half, part, fillers=()):
                # part 0: loads + Q/K t-chunk 2*half, V s-blocks 8*half..+3
                # part 1: Q/K t-chunk 2*half+1, V s-blocks 8*half+4..+7
                fillers = list(fillers)

                def fill():
                    if fillers:
                        fillers.pop(0)()
                tch = 2 * half + part
                xh = []
                for k in range(8):
                    t_ = XK.tile([128, 512], FR, tag=f"xk{k}",
                                 name=f"xk{k}q{tch}")
                    dma(out=t_, in_=xT[128 * k:128 * (k + 1),
                                       512 * tch: 512 * (tch + 1)])
                    xh.append(t_)
                if half == 0 and part == 0:
                    deferred_loads()
                for (wt, qkt, bias, scale) in ((wq_s, qt, bq_s, 0.125),
                                               (wk_s, kt, bk_s, 1.0)):
                    for m in range(2):
                        ps = psA.tile([128, 512], F32, tag="psA")
                        for k in range(8):
                            nc.tensor.matmul(
                                ps,
                                R(wt[:, 256 * k + 128 * m: 256 * k + 128 * m + 128]),
                                R(xh[k]),
                                start=(k == 0), stop=(k == 7))
                        with nc.allow_low_precision(reason="f32r out"):
                            vec.tensor_scalar(
                                qkt[m][:, 512 * tch: 512 * (tch + 1)], ps,
                                scalar1=scale, scalar2=bias[:, m:m + 1],
                                op0=ALU.mult, op1=ALU.add)
                        fill()
                for b_ in range(4):
                    blk = 4 * tch + b_
                    ps = psA.tile([128, 512], F32, tag="psA")
                    for k in range(8):
                        nc.tensor.matmul(
                            ps[:, 0:CD],
                            R(xh[k][:, 128 * b_: 128 * (b_ + 1)]),
                            R(wv_s[:, 256 * k: 256 * (k + 1)]),
                            start=(k == 0), stop=False)
                    nc.tensor.matmul(ps[:, 0:CD], R(one128), R(bv_s),
                                     start=False, stop=True)
                    vec.tensor_copy(
                        v_s[:, 260 * blk: 260 * blk + 260]
                        .rearrange("p (h e) -> p h e", h=HPC)[:, :, 0:64],
                        ps[:, 0:CD].rearrange("p (h e) -> p h e", e=64))
                    fill()
                while fillers:
                    fillers.pop(0)()

            def b2_chunk(h, c):
                """Scores in [s,t], exp, AV+Z fused; updates hc, r_all, lnr[4c:4c+4]."""
                m, po = h // 2, 64 * (h % 2)
                qh = qt[m][po:po + 64, :]
                kh = kt[m][po:po + 64, :]
                qs = qh[:, 512 * c: 512 * (c + 1)]
                av = psAV.tile([65, 512], F32, tag="psAV", name=f"av{h}_{c}")
                work = []
                # paired full blocks -> one [128,1024] psum + one exp
                for a in range(0, 4 * c, 2):
                    sp = psP.tile([128, 1024], F32, tag="psP", name=f"sp{h}_{c}_{a}")
                    for d_ in range(2):
                        nc.tensor.matmul(
                            sp[:, 512 * d_: 512 * (d_ + 1)],
                            R(kh[:, 128 * (a + d_): 128 * (a + d_ + 1)]),
                            R(qs),
                            start=True, stop=True)
                    e = EP.tile([128, 1024], FR, tag="e2", name=f"e{h}_{c}_{a}")
                    act(e, sp, AF.Exp)
                    work.append((a, e[:, 0:512], 0))
                    work.append((a + 1, e[:, 512:1024], 0))
                # diagonal-region singles, column-trimmed
                for r_ in range(4):
                    blk = 4 * c + r_
                    off = 128 * r_
                    sp = psA.tile([128, 512], F32, tag="psA", name=f"sd{h}_{c}_{r_}")
                    nc.tensor.matmul(
                        sp[:, 0:512 - off],
                        R(kh[:, 128 * blk: 128 * (blk + 1)]),
                        R(qh[:, 512 * c + off: 512 * (c + 1)]),
                        start=True, stop=True)
                    e = EP.tile([128, 512], FR, tag="e", name=f"ed{h}_{c}_{r_}")
                    act(e[:, off:512], sp[:, 0:512 - off], AF.Exp)
                    vec.tensor_mul(e[:, off:off + 128], e[:, off:off + 128],
                                   trit_s)
                    work.append((blk, e, off))
                for idx, (blk, e_, off) in enumerate(work):
                    nc.tensor.matmul(
                        av[0:65, off:512],
                        R(v_s[:, 260 * blk + 65 * h: 260 * blk + 65 * h + 65]),
                        R(e_[:, off:512] if e_.shape[-1] == 512 else e_),
                        start=(idx == 0), stop=(idx == len(work) - 1))

                ztmp = SP.tile([1, 512], F32, tag="ztmp", name=f"zt{h}_{c}")
                vec.tensor_copy(ztmp, av[64:65, :])
                for k in range(4):
                    i_tt = 4 * c + k
                    zt = psA.tile([128, 4], F32, tag="psA", name=f"ztp{h}_{c}_{k}")
                    nc.tensor.matmul(zt[:, 0:1], ztmp[0:1, 128 * k:128 * (k + 1)],
                                     id_s[0:1, 0:1], is_transpose=True,
                                     start=True, stop=True)
                    vec.reciprocal(r_all[h][:, i_tt:i_tt + 1], zt[:, 0:1])
                rrow = SP.tile([1, 512], FR, tag="rrow", name=f"rr{h}_{c}")
                with nc.allow_low_precision(reason="f32r is f32-width"):
                    vec.reciprocal(rrow, av[64:65, :])
                rb_ps = psA.tile([64, 512], F32, tag="psA", name=f"rb{h}_{c}")
                nc.tensor.matmul(rb_ps, R(one64), R(rrow), start=True, stop=True)
                rbc = SP.tile([64, 512], F32, tag="rbc", name=f"rbc{h}_{c}")
                vec.tensor_copy(rbc, rb_ps)
                vec.tensor_mul(
                    hc[m][po:po + 64, 512 * c: 512 * (c + 1)],
                    av[0:64, :], rbc)
                act(lnr[h][:, 4 * c:4 * (c + 1)], r_all[h][:, 4 * c:4 * (c + 1)],
                    AF.Ln)

            def b1_tile(h, i):
                m, po = h // 2, 64 * (h % 2)
                qh = qt[m][po:po + 64, :]
                kh = kt[m][po:po + 64, :]
                ncols = (i + 1) * 128
                w_sb = WP.tile([128, 2048], F32, tag="w", name=f"w{h}_{i}")
                qi = qh[:, 128 * i: 128 * (i + 1)]
                for a in range(0, (ncols + 1023) // 1024):
                    base = 1024 * a
                    M = min(1024, ncols - base)
                    sp = psP.tile([128, 1024], F32, tag="psP", name=f"wp{h}_{i}_{a}")
                    for d_ in range(0, M, 512):
                        N = min(512, M - d_)
                        nc.tensor.matmul(
                            sp[:, d_:d_ + N],
                            R(qi),
                            R(kh[:, base + d_: base + d_ + N]),
                            start=True, stop=True)
                    act(w_sb[:, base:base + M], sp[:, 0:M], AF.Exp,
                        bias=lnr[h][:, i:i + 1], scale=1.0)
                vec.tensor_mul(w_sb[:, ncols - 128:ncols],
                               w_sb[:, ncols - 128:ncols], tri_s)
                dma(out=out_w[h, 128 * i: 128 * (i + 1), 0:ncols],
                    in_=w_sb[:, 0:ncols])

            def proj_chunk(c):
                for q in range(4):
                    o_sb = OP.tile([128, D], F32, tag="o", name=f"o{c}_{q}")
                    op = psP.tile([128, 1024], F32, tag="psP", name=f"op{c}_{q}")
                    for n in range(2):
                        for k in range(2):
                            nc.tensor.matmul(
                                op[:, 512 * n: 512 * (n + 1)],
                                R(hc[k][:, 512 * c + 128 * q: 512 * c + 128 * (q + 1)]),
                                R(wo_s[:, 1024 * k + 512 * n: 1024 * k + 512 * (n + 1)]),
                                start=(k == 0), stop=(k == 1))
                    vec.tensor_copy(o_sb, op)
                    dma(out=out_p[128 * (4 * c + q): 128 * (4 * c + q + 1), :],
                        in_=o_sb)

            # ---------- schedule ----------
            def F(h, i):
                return lambda: b1_tile(h, i)

            def C(h, c):
                return lambda: b2_chunk(h, c)

            a_part(0, 0)
            for h in range(HPC):
                b2_chunk(h, 0)
            a_part(0, 1, fillers=[F(h, i) for i in range(0, 3)
                                  for h in range(HPC)])
            for h in range(HPC):
                b2_chunk(h, 1)
            a_part(1, 0, fillers=[F(h, i) for i in (3, 4, 5)
                                  for h in range(HPC)])
            for h in range(HPC):
                b2_chunk(h, 2)
            a_part(1, 1, fillers=[F(h, i) for i in (6, 7, 8)
                                  for h in range(HPC)])
            XK_cm.__exit__(None, None, None)
            for h in range(HPC):
                b2_chunk(h, 3)
                for i in range(9, 12):
                    b1_tile(h, i)
                if h == 1:
                    proj_chunk(0)
                if h == 2:
                    proj_chunk(1)
            for h in range(HPC):
                for i in range(12, 16):
                    b1_tile(h, i)
                if h == 1:
                    proj_chunk(2)
                if h == 2:
                    proj_chunk(3)

    nc.compile()
    return nc


def make_host_inputs(x, Wq, bq, Wk, bk, Wv, bv, Wo):
    tri = np.tril(np.ones((128, 128), dtype=np.float32))
    idm = np.eye(128, dtype=np.float32)

    in_maps = []
    for c in range(8):
        b = c // 4
        r0 = (c % 4) * CD
        in_maps.append({
            "xT": np.ascontiguousarray(x[b].T),
            "wqT": np.ascontiguousarray(Wq[r0:r0 + CD, :].T),
            "wkT": np.ascontiguousarray(Wk[r0:r0 + CD, :].T),
            "wvT": np.ascontiguousarray(Wv[r0:r0 + CD, :].T),
            "woT": np.ascontiguousarray(Wo[:, r0:r0 + CD].T),
            "bq": (bq[r0:r0 + CD] * 0.125).reshape(2, 128).astype(np.float32),
            "bk": bk[r0:r0 + CD].reshape(2, 128).astype(np.float32),
            "bv": bv[r0:r0 + CD].reshape(1, CD).astype(np.float32),
            "tri": tri, "trit": np.ascontiguousarray(tri.T), "idm": idm,
            "onesv": np.ones((128, 128), dtype=np.float32),
        })
    return in_maps


def assemble(results, bo):
    weights = np.empty((B, H, T, T), dtype=np.float32)
    out = np.zeros((B, T, D), dtype=np.float32)
    for c in range(8):
        b = c // 4
        h0 = (c % 4) * HPC
        weights[b, h0:h0 + HPC] = results[c]["out_w"]
        out[b] += results[c]["out_p"]
    out += bo
    return out, weights


_CACHE = {}


def kernel(x, Wq, bq, Wk, bk, Wv, bv, Wo, bo):
    """Full-input entry: shards across 8 NeuronCores, runs the Bass kernel,
    reassembles full outputs. Returns (out [2,2048,1024], weights [2,16,2048,2048])."""
    from concourse.bass_utils import run_bass_kernel_spmd
    x, Wq, bq, Wk, bk, Wv, bv, Wo, bo = (
        np.asarray(a, dtype=np.float32)
        for a in (x, Wq, bq, Wk, bk, Wv, bv, Wo, bo))
    if "nc" not in _CACHE:
        _CACHE["nc"] = build_nc()
    in_maps = make_host_inputs(x, Wq, bq, Wk, bk, Wv, bv, Wo)
    res = run_bass_kernel_spmd(_CACHE["nc"], in_maps, list(range(8)))
    return assemble(res.results, bo)
